# revision 7
# baseline (speedup 1.0000x reference)
"""Trainium2 Bass kernel for CustomConvWithExtra.

out = conv3x3(x, w_main) + b_main + extra, where extra collapses to a 3x3
border-class table T[b,c,clsh,clsw] (conv of a spatially-constant image).

Design (v4):
 - Data parallel: 1 batch image per NeuronCore (B=8 = 8 cores).
 - ONE matmul per output row-pair: stationary lhsT [57,128] block-diagonal
   (row-block per pair: 27 = ci x kh x kw taps -> 64 channels) + 3 fused
   bias/border rows; moving rhs [57,512] im2col patch; PSUM [128,512].
   float32r operands -> full-rate PE, ~620ns/matmul + ~320ns weight load.
 - Patch rows include the kw replication (54 data rows); each row is filled by
   a single-partition [1, C*512] DMA from DRAM xp, which balance_dma_aps
   sprays over all 16 SDMA engines (descriptor->engine is by SBUF partition
   ordinal, so multi-partition narrow DMAs would pile onto engine 0..k).
 - Output: ob [128, C*512] (partition = pair*64+ch), 2 DMAs per chunk
   (64 partitions each -> 64 descriptors balanced over 16 engines).
"""

from contextlib import ExitStack

import numpy as np

import concourse.bass as bass
import concourse.tile as tile
from concourse import bacc, mybir
from concourse.bass_utils import run_bass_kernel_spmd

# Problem shapes (hardcoded per contract)
B, CIN, H, W = 8, 3, 512, 512
COUT, E, KS = 64, 3, 3
NCORES = 8
KP = 57            # patch partitions: 54 = (pair,ci,kh,kw) + indL + indR + ones
C = 16             # row-pairs per chunk
F32R = mybir.dt.float32r
F32 = mybir.dt.float32

_cache: dict = {}


def _build(h: int = H, w: int = W):
    xrow = w + 2
    xh = h + 2
    pairs = h // 2
    c = min(C, pairs)
    nchunk = pairs // c
    assert pairs % c == 0

    nc = bacc.Bacc("TRN2", target_bir_lowering=False, debug=False)
    xp = nc.dram_tensor("xp", [CIN, xh, xrow], F32R, kind="ExternalInput").ap()
    wts = nc.dram_tensor("wts", [3, KP, 128], F32R, kind="ExternalInput").ap()
    stat = nc.dram_tensor("stat", [3, c * w], F32R, kind="ExternalInput").ap()
    out = nc.dram_tensor("out", [COUT, h, w], F32, kind="ExternalOutput").ap()

    PBUFS = 3
    with tile.TileContext(nc) as tc, ExitStack() as ctx:
        wpool = ctx.enter_context(tc.tile_pool(name="wpool", bufs=1))
        ppool = ctx.enter_context(tc.tile_pool(name="ppool", bufs=PBUFS))
        opool = ctx.enter_context(tc.tile_pool(name="opool", bufs=2))
        pspool = ctx.enter_context(tc.tile_pool(name="pspool", bufs=4, space="PSUM"))

        # Stationary weights: wtile[k, v*128+m] = wts[v, k, m]
        wtile = wpool.tile([KP, 3 * 128], F32R)
        nc.sync.dma_start(
            wtile[:, :],
            bass.AP(wts.tensor, 0, [[128, KP], [KP * 128, 3], [1, 128]]),
        )

        # Patch buffers; static rows 54:57 loaded once per physical buffer.
        patch_tiles = []
        for s in range(PBUFS):
            pt = ppool.tile([KP, c * w], F32R, name=f"patch{s}", tag="patch")
            nc.sync.dma_start(pt[54:57, :], stat[:, :])
            patch_tiles.append(pt)

        for ch in range(nchunk):
            pt = patch_tiles[ch % PBUFS]
            h0 = ch * c * 2  # first output row of chunk
            # Fill rows p = pair*27+ci*9+kh*3+kw <- xp[ci, h0+2j+pair+kh, kw:kw+w]
            n = 0
            for pair in range(2):
                for ci in range(CIN):
                    for kh in range(3):
                        for kw in range(3):
                            p = pair * 27 + ci * 9 + kh * 3 + kw
                            src = bass.AP(
                                xp.tensor,
                                (ci * xh + h0 + pair + kh) * xrow + kw,
                                [[2 * xrow, c], [1, w]],
                            )
                            eng = nc.sync if n % 2 == 0 else nc.scalar
                            eng.dma_start(pt[p : p + 1, :], src)
                            n += 1

            ob = opool.tile([128, c * w], F32, name="ob", tag="ob")
            for j in range(c):
                pairidx = ch * c + j
                vrow = 0 if pairidx == 0 else (2 if pairidx == pairs - 1 else 1)
                ps = pspool.tile([128, w], F32, name="ps", tag="ps")
                nc.tensor.matmul(
                    ps[:, :],
                    wtile[:, vrow * 128 : (vrow + 1) * 128],
                    pt[:, j * w : (j + 1) * w],
                    start=True,
                    stop=True,
                )
                if j % 2 == 0:
                    nc.vector.tensor_copy(ob[:, j * w : (j + 1) * w], ps[:, :])
                else:
                    nc.scalar.copy(ob[:, j * w : (j + 1) * w], ps[:, :])

            for pair in range(2):
                dst = bass.AP(
                    out.tensor,
                    (h0 + pair) * w,
                    [[h * w, COUT], [2 * w, c], [1, w]],
                )
                nc.sync.dma_start(dst, ob[pair * 64 : (pair + 1) * 64, :])

    nc.compile()
    return nc


def _host_prep(x, v, wm, bm, we, be, h=H, w=W, c=C):
    """Per-core inputs: padded image, fused weight variants, static patch rows."""
    Bb = x.shape[0]
    vr = v.reshape(Bb, COUT, E).astype(np.float64)

    sets = {0: [1, 2], 1: [0, 1, 2], 2: [0, 1]}
    Mcl = np.zeros((COUT, E, 3, 3), np.float64)
    we64 = we.astype(np.float64)
    for ch_ in range(3):
        for cw in range(3):
            Mcl[:, :, ch_, cw] = we64[:, :, sets[ch_], :][:, :, :, sets[cw]].sum((2, 3))
    T = (
        np.einsum("bce,cehw->bchw", vr, Mcl)
        + bm.astype(np.float64)[None, :, None, None]
        + be.astype(np.float64)[None, :, None, None]
    )

    xp = np.pad(x, ((0, 0), (0, 0), (1, 1), (1, 1))).astype(np.float32)

    # vrow: 0 = pair (rows 0,1) classes (top,mid); 1 = interior; 2 = (mid,bot)
    pair_cls = {0: (0, 1), 1: (1, 1), 2: (1, 2)}
    wts = np.zeros((Bb, 3, KP, 128), np.float32)
    for b in range(Bb):
        for vrow in range(3):
            for pair in range(2):
                cols = slice(pair * 64, pair * 64 + 64)
                for ci in range(CIN):
                    for kh in range(KS):
                        for kw in range(KS):
                            wts[b, vrow, pair * 27 + ci * 9 + kh * 3 + kw, cols] = wm[
                                :, ci, kh, kw
                            ]
                cls = pair_cls[vrow][pair]
                wts[b, vrow, 54, cols] = T[b, :, cls, 0] - T[b, :, cls, 1]
                wts[b, vrow, 55, cols] = T[b, :, cls, 2] - T[b, :, cls, 1]
                wts[b, vrow, 56, cols] = T[b, :, cls, 1]

    stat = np.zeros((3, c * w), np.float32)
    stat[0, 0::w] = 1.0        # w == 0 indicator
    stat[1, w - 1 :: w] = 1.0  # w == W-1 indicator
    stat[2, :] = 1.0           # ones row (base bias)
    return xp, wts, stat


def kernel(**inputs) -> np.ndarray:
    x = np.ascontiguousarray(np.asarray(inputs["x"], np.float32))
    v = np.asarray(inputs["extra_inputs"], np.float32)
    wm = np.asarray(inputs["w_main"], np.float32)
    bm = np.asarray(inputs["b_main"], np.float32)
    we = np.asarray(inputs["w_extra"], np.float32)
    be = np.asarray(inputs["b_extra"], np.float32)

    xp, wts, stat = _host_prep(x, v, wm, bm, we, be)

    if "nc" not in _cache:
        _cache["nc"] = _build()
    nc = _cache["nc"]

    in_maps = [{"xp": xp[b], "wts": wts[b], "stat": stat} for b in range(B)]
    res = run_bass_kernel_spmd(nc, in_maps, list(range(NCORES)))
    return np.stack([res.results[b]["out"] for b in range(B)]).astype(np.float32)


# revision 8
# speedup vs baseline: 1.7878x; 1.7878x over previous
"""Trainium2 Bass kernel for CustomConvWithExtra.

out = conv3x3(x, w_main) + b_main + extra, where extra collapses to a 3x3
border-class table T[b,c,clsh,clsw] (conv of a spatially-constant image).

Design (v3):
 - Data parallel: 1 batch image per NeuronCore (B=8 = 8 cores).
 - Per output ROW-PAIR: 3 accumulating matmuls (one per kw tap column) into a
   single PSUM bank [128,512].  Stationary lhsT [21,128] is block-diagonal
   (row-block per pair: 9 = ci x kh taps -> 64 channels); kw is applied by
   sliding the rhs window along the patch free dim (patch rows are 514 wide).
   float32r operands -> full-rate PE.
 - Patch rows = raw padded-image rows (no kw replication): 18 data rows
   (pair,ci,kh) + 3 static rows (w==0 indicator, w==W-1 indicator, ones) that
   fuse the whole bias/extra term into the kw=1 matmul.
 - Patch fill: per-row [1, C*514] DMAs straight from DRAM xp; balance_dma_aps
   sprays single-partition DMAs across all 16 SDMA engines.
 - Output: ob [128, C*512] (partition = pair*64+ch), 2 DMAs per chunk of
   C row-pairs (64 partitions each -> descriptors balanced over engines).
"""

from contextlib import ExitStack

import numpy as np

import concourse.bass as bass
import concourse.tile as tile
from concourse import bacc, mybir
from concourse.bass_utils import run_bass_kernel_spmd

# Problem shapes (hardcoded per contract)
B, CIN, H, W = 8, 3, 512, 512
COUT, E, KS = 64, 3, 3
NCORES = 8
KP = 21            # patch partitions: 18 = (pair,ci,kh) + indL + indR + ones
C = 16             # row-pairs per chunk
F32R = mybir.dt.float32r
F32 = mybir.dt.float32

_cache: dict = {}


def _build(h: int = H, w: int = W):
    xrow = w + 2
    xh = h + 2
    pairs = h // 2
    c = min(C, pairs)
    nchunk = pairs // c
    assert pairs % c == 0

    nc = bacc.Bacc("TRN2", target_bir_lowering=False, debug=False)
    xp = nc.dram_tensor("xp", [CIN, xh, xrow], F32R, kind="ExternalInput").ap()
    wts = nc.dram_tensor("wts", [9, KP, 128], F32R, kind="ExternalInput").ap()
    stat = nc.dram_tensor("stat", [3, c * xrow], F32R, kind="ExternalInput").ap()
    out = nc.dram_tensor("out", [COUT, h, w], F32, kind="ExternalOutput").ap()

    PBUFS = 4
    with tile.TileContext(nc) as tc, ExitStack() as ctx:
        wpool = ctx.enter_context(tc.tile_pool(name="wpool", bufs=1))
        ppool = ctx.enter_context(tc.tile_pool(name="ppool", bufs=PBUFS))
        opool = ctx.enter_context(tc.tile_pool(name="opool", bufs=2))
        pspool = ctx.enter_context(tc.tile_pool(name="pspool", bufs=8, space="PSUM"))

        # Stationary weights: wtile[k, u*128+m] = wts[u, k, m], u = vrow*3+kw
        wtile = wpool.tile([KP, 9 * 128], F32R)
        nc.sync.dma_start(
            wtile[:, :],
            bass.AP(wts.tensor, 0, [[128, KP], [KP * 128, 9], [1, 128]]),
        )

        # Patch buffers; static rows 18:21 loaded once per physical buffer.
        patch_tiles = []
        for s in range(PBUFS):
            pt = ppool.tile([KP, c * xrow], F32R, name=f"patch{s}", tag="patch")
            nc.sync.dma_start(pt[18:21, :], stat[:, :])
            patch_tiles.append(pt)

        for ch in range(nchunk):
            pt = patch_tiles[ch % PBUFS]
            h0 = ch * c * 2  # first output row of chunk
            # Fill data rows: p = pair*9 + ci*3 + kh <- xp[ci, h0+2j+pair+kh, :]
            n = 0
            for pair in range(2):
                for ci in range(CIN):
                    for kh in range(3):
                        p = pair * 9 + ci * 3 + kh
                        src = bass.AP(
                            xp.tensor,
                            (ci * xh + h0 + pair + kh) * xrow,
                            [[2 * xrow, c], [1, xrow]],
                        )
                        eng = nc.sync if n % 2 == 0 else nc.scalar
                        eng.dma_start(pt[p : p + 1, :], src)
                        n += 1

            ob = opool.tile([128, c * w], F32, name="ob", tag="ob")
            for j in range(c):
                pairidx = ch * c + j
                vrow = 0 if pairidx == 0 else (2 if pairidx == pairs - 1 else 1)
                ps = pspool.tile([128, w], F32, name="ps", tag="ps")
                for kw in range(3):
                    u = vrow * 3 + kw
                    nc.tensor.matmul(
                        ps[:, :],
                        wtile[:, u * 128 : (u + 1) * 128],
                        pt[:, j * xrow + kw : j * xrow + kw + w],
                        start=(kw == 0),
                        stop=(kw == 2),
                    )
                if j % 2 == 0:
                    nc.vector.tensor_copy(ob[:, j * w : (j + 1) * w], ps[:, :])
                else:
                    nc.scalar.copy(ob[:, j * w : (j + 1) * w], ps[:, :])

            for pair in range(2):
                dst = bass.AP(
                    out.tensor,
                    (h0 + pair) * w,
                    [[h * w, COUT], [2 * w, c], [1, w]],
                )
                nc.gpsimd.dma_start(dst, ob[pair * 64 : (pair + 1) * 64, :])

    nc.compile()
    return nc


def _host_prep(x, v, wm, bm, we, be, h=H, w=W, c=C):
    """Per-core inputs: padded image, fused weight variants, static patch rows."""
    Bb = x.shape[0]
    vr = v.reshape(Bb, COUT, E).astype(np.float64)

    sets = {0: [1, 2], 1: [0, 1, 2], 2: [0, 1]}
    Mcl = np.zeros((COUT, E, 3, 3), np.float64)
    we64 = we.astype(np.float64)
    for ch_ in range(3):
        for cw in range(3):
            Mcl[:, :, ch_, cw] = we64[:, :, sets[ch_], :][:, :, :, sets[cw]].sum((2, 3))
    T = (
        np.einsum("bce,cehw->bchw", vr, Mcl)
        + bm.astype(np.float64)[None, :, None, None]
        + be.astype(np.float64)[None, :, None, None]
    )

    xp = np.pad(x, ((0, 0), (0, 0), (1, 1), (1, 1))).astype(np.float32)

    # vrow: 0 = chunk containing output row 0 (pair rows classes (top,mid)),
    #       1 = interior (mid,mid), 2 = last pair (mid,bottom)
    pair_cls = {0: (0, 1), 1: (1, 1), 2: (1, 2)}
    wts = np.zeros((Bb, 9, KP, 128), np.float32)
    for b in range(Bb):
        for vrow in range(3):
            for kw in range(KS):
                u = vrow * 3 + kw
                for pair in range(2):
                    cols = slice(pair * 64, pair * 64 + 64)
                    for ci in range(CIN):
                        for kh in range(KS):
                            wts[b, u, pair * 9 + ci * 3 + kh, cols] = wm[:, ci, kh, kw]
                    if kw == 1:  # statics only fire in the center-kw matmul
                        cls = pair_cls[vrow][pair]
                        wts[b, u, 18, cols] = T[b, :, cls, 0] - T[b, :, cls, 1]
                        wts[b, u, 19, cols] = T[b, :, cls, 2] - T[b, :, cls, 1]
                        wts[b, u, 20, cols] = T[b, :, cls, 1]

    xrow = w + 2
    stat = np.zeros((3, c * xrow), np.float32)
    stat[0, 1::xrow] = 1.0          # rhs col 0 under kw=1 window
    stat[1, w::xrow] = 1.0          # rhs col w-1 under kw=1 window
    stat[2, :] = 1.0                # ones row (base bias)
    return xp, wts, stat


def kernel(**inputs) -> np.ndarray:
    x = np.ascontiguousarray(np.asarray(inputs["x"], np.float32))
    v = np.asarray(inputs["extra_inputs"], np.float32)
    wm = np.asarray(inputs["w_main"], np.float32)
    bm = np.asarray(inputs["b_main"], np.float32)
    we = np.asarray(inputs["w_extra"], np.float32)
    be = np.asarray(inputs["b_extra"], np.float32)

    xp, wts, stat = _host_prep(x, v, wm, bm, we, be)

    if "nc" not in _cache:
        _cache["nc"] = _build()
    nc = _cache["nc"]

    in_maps = [{"xp": xp[b], "wts": wts[b], "stat": stat} for b in range(B)]
    res = run_bass_kernel_spmd(nc, in_maps, list(range(NCORES)))
    return np.stack([res.results[b]["out"] for b in range(B)]).astype(np.float32)
